# revision 5
# baseline (speedup 1.0000x reference)
"""Trainium2 Bass kernel for ChainRelativePositionEmbedding.

Problem: out[0, i, j, :] = Wt[1 + ridx_finl(i,j)] + same_chain(i,j) * Wt[0] + bias
with 3 chains of 512 residues (L = 1536), Wt = weight.T [67, 128].

Every output pair-vector is one of only 66 distinct 128-float vectors:
  same chain:  T_same[k] = Wt[1+k] + Wt[0] + bias,  k = clip(p_i - p_j + 32, 0, 64)
  cross chain: T_diff    = Wt[66] + bias

So the kernel is pure DMA replication out of tiny SBUF-resident tables - no
arithmetic at all. Work is sharded across 8 cores with an INTERLEAVED row
assignment (core c owns global rows i == c (mod 8)), which makes the Bass
program identical on every core:
  * local row r in [0,192): chain b = r//64, rp = r%64, residue p = 8*rp + c.
  * the same-chain block of row r is a 512-entry sliding window into a
    1024-entry "master" strip (content is per-core host-built from
    weight/bias): out[r, 512b+8q+s, :] = M[8*(63-rp+q)+s], where the strip
    lives in SBUF as msb[p, s*128:(s+1)*128] = M[8p+s].

HW-profiled bottleneck this version removes: every dma_start's trailing
semaphore descriptor waits for an HBM write receipt (~2 us at load), and a
stalled SDMA engine can NOT switch queues mid-packet, so a row-per-DMA job
list (192 x 256 KiB) caps at ~100 GB/s on its ring and ~330 GB/s total
(~505 us).  Fix: spend 12 MiB of SBUF to materialize every row's window
DISJOINTLY, collapsing the 192 row DMAs into 6 large DMAs:

  W[96 partitions, 32 blocks x 1024] with W[q, v*1024+sd] = M[8*(31-v+q)+s]
    = 32 partition-shifted SBUF->SBUF copies  W[0:96, v-block] = msb[31-v:127-v, :]
    (fast ~0.3 us SBUF receipts, off the HBM path, ~25 us total).
  Rows rp in [32,64) read W[0:64, :], rows rp in [0,32) read the SAME strip
  at partition base 32 (W[32:96, :]) - the half-offset makes one 12 MiB strip
  serve all 64 rows.  Each chain-half is then ONE DMA:
    src  W[32h':32h'+64, :]        -> balanced [[32768,64],[1024,32],[1,1024]]
    dst  raw AP [[1024,64],[196608,32],[1,1024]] (q outer, rp mid, 4 KiB runs)

The 96 MiB of cross-chain T_diff replication stays 4 giant DMAs from a small
constant tile using a step-0 (broadcast) middle dim (HW-validated).  Final
job list: 2 loads + 32 SBUF builds + 4 const + 6 diag DMAs; only 12 HBM-
receipt stalls total vs 196.  Roofline: 151 MiB/core of HBM writes at the
~358 GB/s per-core HBM bound => ~422 us.
"""

import numpy as np

import concourse.bass as bass
import concourse.mybir as mybir
from concourse.ap import AP as RawAP
from concourse.bass_utils import run_bass_kernel_spmd

L = 1536          # total residues (3 chains x 512)
D = 128           # embedding dim
NCORES = 8
RPC = L // NCORES  # rows per core = 192

# Module-level knobs/results (used by test.py; harness just calls kernel()).
TRACE = False
TRACE_KWARGS = {}
LAST_RESULTS = None

_CACHED_NC = None


def _build_nc():
    nc = bass.Bass()
    f32 = mybir.dt.float32

    master = nc.declare_dram_parameter("master", [128, 1024], f32, isOutput=False)
    constsrc = nc.declare_dram_parameter("constsrc", [128, 1024], f32, isOutput=False)
    out = nc.declare_dram_parameter("out", [RPC, L, D], f32, isOutput=True)

    with (
        nc.sbuf_tensor("msb", [128, 1024], f32) as msb,
        nc.sbuf_tensor("csb", [128, 1024], f32) as csb,
        nc.sbuf_tensor("W", [96, 32 * 1024], f32) as W,
        nc.semaphore("dsem") as dsem,
        nc.semaphore("bsem") as bsem,
        nc.Block() as block,
    ):
        out_h = out[:, :, :].tensor

        def cbc(reps):
            return csb[:, :].unsqueeze(1).broadcast_to([128, reps, 1024])

        const_jobs = [
            (out[0:64, 512:1536, :], cbc(64)),     # chain 0 rows: j in [512,1536)
            (out[64:128, 0:512, :], cbc(32)),      # chain 1 rows: j in [0,512)
            (out[64:128, 1024:1536, :], cbc(32)),  # chain 1 rows: j in [1024,1536)
            (out[128:192, 0:1024, :], cbc(64)),    # chain 2 rows: j in [0,1024)
        ]

        # Strip build: W[0:96, v-block] = msb[31-v : 127-v, :]  (v = 0..31)
        build_jobs = [
            (W[0:96, 1024 * v : 1024 * (v + 1)], msb[31 - v : 127 - v, :])
            for v in range(32)
        ]

        # Diag DMAs: chain b, rows rp in [32h', 32h'+32) read W at partition
        # base 32*(1-h').  dst iterates (q, rp, s*128+d): element (q, v, sd)
        # lands at out[64b+32h'+v, 512b+8q+s, d].
        diag_jobs = []
        for b in range(3):
            for hp in (0, 1):  # hp=0: rows [0,32) <- W[32:96]; hp=1: rows [32,64) <- W[0:64]
                off = (64 * b + 32 * hp) * (L * D) + (512 * b) * D
                dst = RawAP(out_h, off, [[1024, 64], [L * D, 32], [1, 1024]])
                src = W[32 * (1 - hp) : 32 * (1 - hp) + 64, :]
                diag_jobs.append((dst, src))

        # dsem: 2 loads + 4 const + 6 diag = 12 DMAs x 16
        total_incs = 16 * (2 + len(const_jobs) + len(diag_jobs))
        build_incs = 16 * len(build_jobs)

        @block.sync
        def _(eng):
            eng.dma_start(out=msb[:, :], in_=master[:, :]).then_inc(dsem, 16)
            eng.dma_start(out=csb[:, :], in_=constsrc[:, :]).then_inc(dsem, 16)
            eng.wait_ge(dsem, 32)
            for dst, src in const_jobs:
                eng.dma_start(out=dst, in_=src).then_inc(dsem, 16)
            eng.wait_ge(dsem, total_incs)

        @block.scalar
        def _(eng):
            eng.wait_ge(dsem, 32)
            for dst, src in build_jobs:
                eng.dma_start(out=dst, in_=src).then_inc(bsem, 16)
            eng.wait_ge(bsem, build_incs)
            for dst, src in diag_jobs:
                eng.dma_start(out=dst, in_=src).then_inc(dsem, 16)

    return nc


def _expected_asym_id():
    return np.repeat(np.arange(1, 4, dtype=np.int32), 512)


def _fallback_numpy(lengths, asym_id, weight, bias):
    """Generic host path if inputs ever deviate from the hardcoded structure."""
    lengths = np.asarray(lengths).astype(np.int64)
    asym_id = np.asarray(asym_id)
    weight = np.asarray(weight, np.float32)
    bias = np.asarray(bias, np.float32)
    ridx_max = (weight.shape[1] - 3) // 2
    idxs = np.concatenate([np.arange(int(l), dtype=np.int32) for l in lengths])
    asym_mat = asym_id[:, None] == asym_id[None, :]
    ridx = idxs[:, None] - idxs[None, :]
    ridx_clip = np.clip(ridx + ridx_max, 0, 2 * ridx_max)
    ridx_finl = np.where(asym_mat, ridx_clip, 2 * ridx_max + 1)
    Wt = weight.T
    pfea = Wt[1 + ridx_finl] + asym_mat.astype(weight.dtype)[..., None] * Wt[0] + bias
    return pfea[None]


def kernel(lengths=None, asym_id=None, weight=None, bias=None):
    global _CACHED_NC, LAST_RESULTS

    lengths = np.asarray(lengths)
    asym_id = np.asarray(asym_id)
    weight = np.asarray(weight, np.float32)
    bias = np.asarray(bias, np.float32)

    if (
        weight.shape != (D, 67)
        or tuple(lengths.astype(np.int64)) != (512, 512, 512)
        or asym_id.shape != (L,)
        or not np.array_equal(asym_id, _expected_asym_id())
    ):
        return _fallback_numpy(lengths, asym_id, weight, bias)

    # Combined lookup tables (same float op order as the reference).
    Wt = weight.T                           # [67, 128]
    T_same = Wt[1:66] + Wt[0] + bias        # [65, 128]
    T_diff = (Wt[66] + bias).astype(np.float32)  # [128]

    # Per-core master strip: master_c[u] = T_same[clip(543 + c - u, 0, 64)],
    # laid out [partition p, vector f] with u = 7 + 8p + f.
    u = 7 + 8 * np.arange(128)[:, None] + np.arange(8)[None, :]  # [128, 8]
    const_np = np.ascontiguousarray(np.tile(T_diff, (128, 8)))  # [128, 1024]

    in_maps = []
    for c in range(NCORES):
        idx = np.clip(543 + c - u, 0, 64)
        master_np = np.ascontiguousarray(T_same[idx].reshape(128, 1024))
        in_maps.append({"master": master_np, "constsrc": const_np})

    if _CACHED_NC is None:
        _CACHED_NC = _build_nc()

    res = run_bass_kernel_spmd(
        _CACHED_NC,
        in_maps,
        list(range(NCORES)),
        trace=TRACE,
        **TRACE_KWARGS,
    )
    LAST_RESULTS = res

    full = np.empty((L, L, D), np.float32)
    for c in range(NCORES):
        full[c::8] = res.results[c]["out"]
    return full[None]
